# revision 1
# baseline (speedup 1.0000x reference)
"""2-layer GCN (GridGNN) on 8 Trainium2 NeuronCores.

2D sharding: core c=(q,h), q=c//2 source-quarter (25088 nodes), h=c%2
destination parity group. Core c handles edges with src in quarter q and
dst in shards {s: s%2==h}. Messages gathered via dma_gather (int16) from
a per-quarter fp32 table in HBM; scatter-reduce onto 128-node destination
windows via one-hot matmuls on the PE; partial aggregates ReduceScattered
within parity groups; inter-layer halo via pairwise AllGather; pooled
sums AllReduced; linear+softmax head on device.
"""
import numpy as np
import ml_dtypes

N_NODES = 100000
N_GRAPHS = 64
F = 64
N_ACT = 3
P = 128
SHARD = 12544
NW = 98
QUART = 2 * SHARD
QT = 196
ZROW = 196            # zero row: r = p*197+t with p=0, t=196
NWIN = 4 * NW
CHUNK_W = 16

bf16 = ml_dtypes.bfloat16


def _prep(x, edge_index, batch, W1, b1, W2, b2, Wl, bl):
    src = edge_index[0].astype(np.int64)
    dst = edge_index[1].astype(np.int64)
    q_e = src // QUART
    shard_e = dst // SHARD
    core_e = q_e * 2 + (shard_e % 2)

    per_core = []
    cnts = np.zeros((8, NWIN), np.int64)
    for c in range(8):
        m = core_e == c
        s, d = src[m], dst[m]
        sh = d // SHARD
        wgid = (sh // 2) * NW + (d - sh * SHARD) // P
        order = np.argsort(wgid, kind="stable")
        s, d, wgid = s[order], d[order], wgid[order]
        dloc = (d - (d // SHARD) * SHARD) % P
        sl = s - (c // 2) * QUART
        ridx = (sl % P) * (QT + 1) + sl // P
        np.add.at(cnts[c], wgid, 1)
        per_core.append((ridx.astype(np.int16), dloc, wgid))

    T_w = np.ceil(cnts.max(axis=0) / P).astype(np.int64)
    Etot = int(T_w.sum()) * P
    offs = np.concatenate([[0], np.cumsum(T_w * P)]).astype(np.int64)

    idx_all = np.full((8, Etot), ZROW, np.int16)
    dst_all = np.zeros((8, Etot), np.float32)
    for c in range(8):
        ridx, dloc, wgid = per_core[c]
        pos = np.searchsorted(wgid, np.arange(NWIN))
        pos_end = np.searchsorted(wgid, np.arange(NWIN), side="right")
        for w in range(NWIN):
            n = pos_end[w] - pos[w]
            idx_all[c, offs[w]:offs[w] + n] = ridx[pos[w]:pos_end[w]]
            dst_all[c, offs[w]:offs[w] + n] = dloc[pos[w]:pos_end[w]]

    chunks = []
    w0 = 0
    while w0 < NWIN:
        w1 = min(w0 + CHUNK_W, NWIN)
        chunks.append((w0, w1, int(offs[w0]), int(offs[w1])))
        w0 = w1
    idx_sb = []
    for c in range(8):
        cols = []
        for (_, _, a, b) in chunks:
            seg = idx_all[c, a:b].reshape(-1, 16).T
            cols.append(np.tile(seg, (8, 1)))
        idx_sb.append(np.concatenate(cols, axis=1))
    idx_sb = np.stack(idx_sb)
    dst_sb = np.ascontiguousarray(
        dst_all.reshape(8, -1, P).transpose(0, 2, 1).astype(bf16))

    deg = np.zeros(8 * SHARD, np.int64)
    np.add.at(deg, dst, 1)
    xpad = np.zeros((8 * SHARD, F), np.float32)
    xpad[:N_NODES] = x
    bpad = np.full(8 * SHARD, 127, np.float32)
    bpad[:N_NODES] = batch

    in_maps = []
    for c in range(8):
        q = c // 2
        qs = slice(q * QUART, (q + 1) * QUART)
        os_ = slice(c * SHARD, (c + 1) * SHARD)
        in_maps.append({
            "xq_T": np.ascontiguousarray(xpad[qs].T.astype(bf16)),
            "xo_T": np.ascontiguousarray(xpad[os_].T.astype(bf16)),
            "degq": np.ascontiguousarray(
                deg[qs].astype(np.float32).reshape(QT, P).T),
            "dego": np.ascontiguousarray(
                deg[os_].astype(np.float32).reshape(NW, P).T),
            "batl": np.ascontiguousarray(
                bpad[os_].reshape(NW, P).T.astype(bf16)),
            "idxs": np.ascontiguousarray(idx_sb[c]),
            "dstl": dst_sb[c],
            "W1": np.ascontiguousarray(W1.astype(bf16)),
            "W2": np.ascontiguousarray(W2.astype(bf16)),
            "b1r": np.broadcast_to(b1, (P, F)).astype(bf16).copy(),
            "b2r": np.broadcast_to(b2, (P, F)).astype(bf16).copy(),
            "Wla": _wl_aug(Wl, bl),
        })
    return in_maps, T_w, chunks


def _wl_aug(Wl, bl):
    Wl_aug = np.zeros((F + 1, 4), np.float32)
    Wl_aug[:F, :3] = Wl
    Wl_aug[F, :3] = bl
    Wl_aug[F, 3] = 1.0
    return Wl_aug


def _build(T_w, chunks):
    import concourse.bass as bass
    import concourse.bacc as bacc
    import concourse.tile as tile
    import concourse.mybir as mybir
    from concourse.library_config import mlp
    from concourse.masks import make_identity

    Etot = int(T_w.sum()) * P
    nc = bacc.Bacc("TRN2", target_bir_lowering=False, debug=False,
                   num_devices=8)
    F32, BF, I16 = mybir.dt.float32, mybir.dt.bfloat16, mybir.dt.int16
    AF = mybir.ActivationFunctionType
    OP = mybir.AluOpType

    def ein(name, shape, dt):
        return nc.dram_tensor(name, shape, dt, kind="ExternalInput")

    xq_T = ein("xq_T", [F, QUART], BF)
    xo_T = ein("xo_T", [F, SHARD], BF)
    degq = ein("degq", [P, QT], F32)
    dego = ein("dego", [P, NW], F32)
    batl = ein("batl", [P, NW], BF)
    idxs = ein("idxs", [P, Etot // 16], I16)
    dstl = ein("dstl", [P, Etot // P], BF)
    W1h = ein("W1", [F, F], BF)
    W2h = ein("W2", [F, F], BF)
    b1h = ein("b1r", [P, F], BF)
    b2h = ein("b2r", [P, F], BF)
    Wlh = ein("Wla", [F + 1, 4], F32)
    out_h = nc.dram_tensor("out", [N_GRAPHS, N_ACT], F32,
                           kind="ExternalOutput")

    subt = [nc.dram_tensor(f"sub{i}", [P * (QT + 1), F], F32, kind="Internal")
            for i in range(2)]
    rs_in = [nc.dram_tensor(f"rs_in{i}", [4 * SHARD, F], BF, kind="Internal")
             for i in range(2)]
    rs_out = [nc.dram_tensor(f"rs_out{i}", [SHARD, F], BF, kind="Internal")
              for i in range(2)]
    ag_in = nc.dram_tensor("ag_in", [SHARD, F], BF, kind="Internal")
    ag_out = nc.dram_tensor("ag_out", [QUART, F], BF, kind="Internal")
    pool_in = nc.dram_tensor("pool_in", [F + 1, N_GRAPHS], F32,
                             kind="Internal")
    pool_out = nc.dram_tensor("pool_out", [F + 1, N_GRAPHS], F32,
                              kind="Internal", addr_space="Shared")

    RG2 = [[0, 1], [2, 3], [4, 5], [6, 7]]
    RGH = [[0, 2, 4, 6], [1, 3, 5, 7]]
    RG8 = [[0, 1, 2, 3, 4, 5, 6, 7]]

    nc.gpsimd.load_library(mlp)
    with tile.TileContext(nc) as tc:
        with tc.tile_pool(name="cst", bufs=1) as cst, \
             tc.tile_pool(name="big", bufs=1) as big, \
             tc.tile_pool(name="mv", bufs=2) as mv, \
             tc.tile_pool(name="oh", bufs=4) as ohp, \
             tc.tile_pool(name="ps", bufs=2, space="PSUM") as ps, \
             tc.tile_pool(name="pw", bufs=2, space="PSUM") as pw, \
             tc.tile_pool(name="pc", bufs=1, space="PSUM") as pc:

            ident = cst.tile([P, P], BF)
            make_identity(nc, ident[:])
            iota_i = cst.tile([P, P], mybir.dt.int32)
            nc.gpsimd.iota(iota_i[:], pattern=[[1, P]], base=0,
                           channel_multiplier=0)
            iota = cst.tile([P, P], BF)
            nc.vector.tensor_copy(out=iota[:], in_=iota_i[:])

            W1t = cst.tile([F, F], BF)
            nc.sync.dma_start(out=W1t[:], in_=W1h.ap())
            W2t = cst.tile([F, F], BF)
            nc.sync.dma_start(out=W2t[:], in_=W2h.ap())
            b1t = cst.tile([P, F], BF)
            nc.sync.dma_start(out=b1t[:], in_=b1h.ap())
            b2t = cst.tile([P, F], BF)
            nc.sync.dma_start(out=b2t[:], in_=b2h.ap())
            batt = cst.tile([P, NW], BF)
            nc.sync.dma_start(out=batt[:], in_=batl.ap())
            idxt = cst.tile([P, Etot // 16], I16)
            nc.sync.dma_start(out=idxt[:], in_=idxs.ap())
            dstt = cst.tile([P, Etot // P], BF)
            nc.sync.dma_start(out=dstt[:], in_=dstl.ap())


            def make_dinv(src_h, n):
                t = cst.tile([P, n], F32, tag=f"dinv{n}")
                nc.sync.dma_start(out=t[:], in_=src_h.ap())
                nc.vector.tensor_scalar(out=t[:], in0=t[:], scalar1=1.0,
                                        scalar2=None, op0=OP.add)
                nc.vector.reciprocal(out=t[:], in_=t[:])
                nc.scalar.activation(t[:], t[:], AF.Sqrt)
                return t

            dinvq = make_dinv(degq, QT)
            dinvo = make_dinv(dego, NW)

            stag = big.tile([P, (QT + 1) * F], BF)
            nc.vector.memset(stag[:, QT * F:], 0.0)
            tso = big.tile([P, NW * F], BF)      # tscaled1 own
            h1own = big.tile([P, NW * F], BF)
            self2 = big.tile([P, NW * F], BF)
            h2aug = big.tile([P, NW * (F + 1)], BF)

            s3q = stag[:].rearrange("p (t f) -> p t f", f=F)

            # ---- layer 1 transform (quarter, shard-asc), streamed ----
            XC = 28
            tso3 = tso[:].rearrange("p (t f) -> p t f", f=F)
            for xh, nt_, dv_, o3 in ((xq_T, QT, dinvq, s3q),
                                     (xo_T, NW, dinvo, tso3)):
                for t0 in range(0, nt_, XC):
                    t1 = min(t0 + XC, nt_)
                    xc = mv.tile([F, XC * P], BF, tag="xc")
                    nc.sync.dma_start(out=xc[:, :(t1 - t0) * P],
                                      in_=xh.ap()[:, t0 * P:t1 * P])
                    for t in range(t0, t1):
                        pt = pw.tile([P, F], F32, space="PSUM", tag="tr")
                        nc.tensor.matmul(
                            out=pt[:], lhsT=xc[:, (t - t0) * P:(t - t0 + 1) * P],
                            rhs=W1t[:], start=True, stop=True)
                        nc.vector.tensor_tensor(
                            out=o3[:, t, :], in0=pt[:],
                            in1=dv_[:, t:t + 1].to_broadcast([P, F]),
                            op=OP.mult)
            nc.gpsimd.dma_start(
                out=subt[0].ap().rearrange("(p t) f -> p t f", p=P),
                in_=stag[:].rearrange("p (t f) -> p t f", f=F))

            MSZ = max((b - a) // P for (_, _, a, b) in chunks)
            def edge_phase(li):
                for (w0, w1, a, b) in chunks:
                    nt = (b - a) // P
                    cpart = mv.tile([P, CHUNK_W * F], BF, tag="cpart")
                    nc.vector.memset(cpart[:], 0.0)
                    cp3 = cpart[:].rearrange("p (w f) -> p w f", f=F)
                    msg = mv.tile([P, MSZ * F], F32, tag="msg")
                    nc.gpsimd.dma_gather(
                        out_ap=msg[:, :nt * F].rearrange(
                            "p (t f) -> p t f", f=F),
                        in_ap=subt[li].ap(),
                        idxs_ap=idxt[:, a // 16:b // 16],
                        num_idxs=b - a,
                        num_idxs_reg=b - a,
                        elem_size=F,
                        single_packet=False,
                    )
                    ti = 0
                    for w in range(w0, w1):
                        tw = int(T_w[w])
                        if tw == 0:
                            continue
                        oht = ohp.tile([P, 8 * P], F32, tag="oh")
                        nc.vector.tensor_tensor(
                            out=oht[:, :tw * P].rearrange(
                                "p (t j) -> p t j", j=P),
                            in0=dstt[:, (a // P) + ti:(a // P) + ti + tw]
                                .unsqueeze(2).to_broadcast([P, tw, P]),
                            in1=iota[:].unsqueeze(1).to_broadcast([P, tw, P]),
                            op=OP.is_equal)
                        acc = ps.tile([P, F], F32, space="PSUM", tag="acc")
                        for k in range(tw):
                            nc.tensor.matmul(
                                out=acc[:],
                                lhsT=oht[:, k * P:(k + 1) * P],
                                rhs=msg[:, (ti + k) * F:(ti + k + 1) * F],
                                start=(k == 0), stop=(k == tw - 1))
                        nc.vector.tensor_copy(out=cp3[:, w - w0, :],
                                              in_=acc[:])
                        ti += tw
                    nc.sync.dma_start(
                        out=rs_in[li].ap()[w0 * P:w1 * P, :].rearrange(
                            "(w p) f -> p w f", p=P),
                        in_=cpart[:, :(w1 - w0) * F].rearrange(
                            "p (w f) -> p w f", f=F))
                nc.gpsimd.collective_compute(
                    "ReduceScatter", OP.add, replica_groups=RGH,
                    ins=[rs_in[li].ap()], outs=[rs_out[li].ap()])

            # ---- layer 1 ----
            edge_phase(0)
            agg1 = big.tile([P, NW * F], BF, tag="agg")
            nc.sync.dma_start(
                out=agg1[:].rearrange("p (w f) -> p w f", f=F),
                in_=rs_out[0].ap().rearrange("(w p) f -> p w f", p=P))
            a3 = agg1[:].rearrange("p (w f) -> p w f", f=F)
            h3 = h1own[:].rearrange("p (w f) -> p w f", f=F)
            # h1 = relu((agg + tscaled1_own) * dinv + b1)
            for w in range(NW):
                dv = dinvo[:, w:w + 1].to_broadcast([P, F])
                nc.vector.tensor_tensor(out=h3[:, w, :], in0=a3[:, w, :],
                                        in1=tso3[:, w, :], op=OP.add)
                nc.vector.tensor_tensor(out=h3[:, w, :], in0=h3[:, w, :],
                                        in1=dv, op=OP.mult)
                nc.vector.tensor_tensor(out=h3[:, w, :], in0=h3[:, w, :],
                                        in1=b1t[:], op=OP.add)
                nc.vector.tensor_scalar(out=h3[:, w, :], in0=h3[:, w, :],
                                        scalar1=0.0, scalar2=None,
                                        op0=OP.max)

            # ---- layer 2 transform (own shard) + self2 ----
            s23 = self2[:].rearrange("p (w f) -> p w f", f=F)
            for w in range(NW):
                trp = pc.tile([P, P], BF, space="PSUM", tag="trp")
                nc.tensor.transpose(out=trp[:F, :], in_=h3[:, w, :],
                                    identity=ident[:])
                h1T = mv.tile([F, P], BF, tag="h1T")
                nc.vector.tensor_copy(out=h1T[:], in_=trp[:F, :])
                pt = pw.tile([P, F], F32, space="PSUM", tag="tr")
                nc.tensor.matmul(out=pt[:], lhsT=h1T[:], rhs=W2t[:],
                                 start=True, stop=True)
                dv = dinvo[:, w:w + 1].to_broadcast([P, F])
                ts2 = mv.tile([P, F], BF, tag="ts2")
                nc.vector.tensor_tensor(out=ts2[:], in0=pt[:], in1=dv,
                                        op=OP.mult)
                nc.vector.tensor_tensor(out=s23[:, w, :], in0=ts2[:], in1=dv,
                                        op=OP.mult)
                nc.sync.dma_start(
                    out=ag_in.ap()[w * P:(w + 1) * P, :], in_=ts2[:])
            nc.gpsimd.collective_compute(
                "AllGather", OP.bypass, replica_groups=RG2,
                ins=[ag_in.ap()], outs=[ag_out.ap()])
            # rebuild staging (bf16) from ag_out, then cast-DMA to subtable2
            nc.sync.dma_start(
                out=stag[:, :QT * F].rearrange("p (t f) -> p t f", f=F),
                in_=ag_out.ap().rearrange("(t p) f -> p t f", p=P))
            nc.gpsimd.dma_start(
                out=subt[1].ap().rearrange("(p t) f -> p t f", p=P),
                in_=stag[:].rearrange("p (t f) -> p t f", f=F))

            # ---- layer 2 ----
            edge_phase(1)
            agg2 = big.tile([P, NW * F], BF, tag="agg")
            nc.sync.dma_start(
                out=agg2[:].rearrange("p (w f) -> p w f", f=F),
                in_=rs_out[1].ap().rearrange("(w p) f -> p w f", p=P))
            a23 = agg2[:].rearrange("p (w f) -> p w f", f=F)
            h2a3 = h2aug[:].rearrange("p (w g) -> p w g", g=F + 1)
            nc.vector.memset(h2aug[:], 1.0)
            for w in range(NW):
                dv = dinvo[:, w:w + 1].to_broadcast([P, F])
                nc.vector.tensor_tensor(out=h2a3[:, w, :F], in0=a23[:, w, :],
                                        in1=dv, op=OP.mult)
                nc.vector.tensor_tensor(out=h2a3[:, w, :F],
                                        in0=h2a3[:, w, :F],
                                        in1=s23[:, w, :], op=OP.add)
                nc.vector.tensor_tensor(out=h2a3[:, w, :F],
                                        in0=h2a3[:, w, :F],
                                        in1=b2t[:], op=OP.add)

            # ---- pooling ----
            poolp = pc.tile([F + 1, N_GRAPHS], F32, space="PSUM", tag="pool")
            for w in range(NW):
                ohg = ohp.tile([P, N_GRAPHS], BF, tag="ohg")
                nc.vector.tensor_tensor(
                    out=ohg[:],
                    in0=batt[:, w:w + 1].to_broadcast([P, N_GRAPHS]),
                    in1=iota[:, :N_GRAPHS], op=OP.is_equal)
                nc.tensor.matmul(out=poolp[:], lhsT=h2a3[:, w, :],
                                 rhs=ohg[:], start=(w == 0),
                                 stop=(w == NW - 1))
            pools = cst.tile([F + 1, N_GRAPHS], F32)
            nc.vector.tensor_copy(out=pools[:], in_=poolp[:])
            nc.sync.dma_start(out=pool_in.ap(), in_=pools[:])
            nc.gpsimd.collective_compute(
                "AllReduce", OP.add, replica_groups=RG8,
                ins=[pool_in.ap()], outs=[pool_out.ap()])

            # ---- head ----
            pooled = cst.tile([F + 1, N_GRAPHS], F32)
            nc.sync.dma_start(out=pooled[:], in_=pool_out.ap())
            Wlt = cst.tile([F + 1, 4], F32)
            nc.sync.dma_start(out=Wlt[:], in_=Wlh.ap())
            zp = pc.tile([4, N_GRAPHS], F32, space="PSUM", tag="z")
            nc.tensor.matmul(out=zp[:], lhsT=Wlt[:], rhs=pooled[:],
                             start=True, stop=True)
            zs = cst.tile([4, N_GRAPHS], F32)
            nc.vector.tensor_copy(out=zs[:], in_=zp[:])
            identf = cst.tile([P, P], F32)
            make_identity(nc, identf[:])
            ztp = pc.tile([N_GRAPHS, 4], F32, space="PSUM", tag="zt")
            nc.tensor.transpose(out=ztp[:], in_=zs[:], identity=identf[:4, :4])
            zt = cst.tile([N_GRAPHS, 4], F32)
            nc.vector.tensor_copy(out=zt[:], in_=ztp[:])
            rc = cst.tile([N_GRAPHS, 1], F32)
            nc.vector.reciprocal(out=rc[:], in_=zt[:, 3:4])
            lg = cst.tile([N_GRAPHS, N_ACT], F32)
            nc.vector.tensor_tensor(out=lg[:], in0=zt[:, :N_ACT],
                                    in1=rc[:].to_broadcast([N_GRAPHS, N_ACT]),
                                    op=OP.mult)
            mx = cst.tile([N_GRAPHS, 1], F32)
            nc.vector.tensor_reduce(out=mx[:], in_=lg[:], op=OP.max, axis=mybir.AxisListType.X)
            nc.vector.tensor_tensor(
                out=lg[:], in0=lg[:],
                in1=mx[:].to_broadcast([N_GRAPHS, N_ACT]), op=OP.subtract)
            nc.scalar.activation(lg[:], lg[:], AF.Exp)
            sm = cst.tile([N_GRAPHS, 1], F32)
            nc.vector.tensor_reduce(out=sm[:], in_=lg[:], op=OP.add, axis=mybir.AxisListType.X)
            nc.vector.reciprocal(out=sm[:], in_=sm[:])
            nc.vector.tensor_tensor(
                out=lg[:], in0=lg[:],
                in1=sm[:].to_broadcast([N_GRAPHS, N_ACT]), op=OP.mult)
            nc.sync.dma_start(out=out_h.ap(), in_=lg[:])

    nc.compile()
    return nc


def kernel(x, edge_index, batch, W1, b1, W2, b2, Wl, bl):
    from concourse.bass_utils import run_bass_kernel_spmd
    in_maps, T_w, chunks = _prep(np.asarray(x), np.asarray(edge_index),
                                 np.asarray(batch), np.asarray(W1),
                                 np.asarray(b1), np.asarray(W2),
                                 np.asarray(b2), np.asarray(Wl),
                                 np.asarray(bl))
    nc = _build(T_w, chunks)
    res = run_bass_kernel_spmd(nc, in_maps, core_ids=list(range(8)))
    return np.asarray(res.results[0]["out"], dtype=np.float32)



# revision 2
# speedup vs baseline: 15.0273x; 15.0273x over previous
"""2-layer GCN (GridGNN) on 8 Trainium2 NeuronCores.

2D sharding: core c=(q,h), q=c//2 source-quarter (25088 nodes), h=c%2
destination parity group. Core c handles edges with src in quarter q and
dst in shards {s: s%2==h}. Each core ships only its OWN shard of x;
the per-quarter staging table is built on-device by transforming the own
shard and AllGathering within quarter pairs (both layers symmetric).
Messages gathered via dma_gather (compact int16 indices, replicated
across partitions on-device) from a per-quarter fp32 table in HBM;
scatter-reduce onto 128-node destination windows via one-hot matmuls on
the PE; partial aggregates ReduceScattered within parity groups; pooled
sums AllReduced; linear+softmax head on device.
"""
import numpy as np
import ml_dtypes

N_NODES = 100000
N_GRAPHS = 64
F = 64
N_ACT = 3
P = 128
SHARD = 12544
NW = 98
QUART = 2 * SHARD
QT = 196
ZROW = 196            # zero row: r = p*197+t with p=0, t=196
NWIN = 4 * NW
CHUNK_W = 16

bf16 = ml_dtypes.bfloat16


def _prep(x, edge_index, batch, W1, b1, W2, b2, Wl, bl):
    src = edge_index[0].astype(np.int64)
    dst = edge_index[1].astype(np.int64)
    q_e = src // QUART
    shard_e = dst // SHARD
    core_e = q_e * 2 + (shard_e % 2)

    per_core = []
    cnts = np.zeros((8, NWIN), np.int64)
    for c in range(8):
        m = core_e == c
        s, d = src[m], dst[m]
        sh = d // SHARD
        wgid = (sh // 2) * NW + (d - sh * SHARD) // P
        order = np.argsort(wgid, kind="stable")
        s, d, wgid = s[order], d[order], wgid[order]
        dloc = (d - (d // SHARD) * SHARD) % P
        sl = s - (c // 2) * QUART
        ridx = (sl % P) * (QT + 1) + sl // P
        np.add.at(cnts[c], wgid, 1)
        per_core.append((ridx.astype(np.int16), dloc, wgid))

    T_w = np.ceil(cnts.max(axis=0) / P).astype(np.int64)
    Etot = int(T_w.sum()) * P
    offs = np.concatenate([[0], np.cumsum(T_w * P)]).astype(np.int64)

    idx_all = np.full((8, Etot), ZROW, np.int16)
    dst_all = np.zeros((8, Etot), np.int8)
    for c in range(8):
        ridx, dloc, wgid = per_core[c]
        pos = np.searchsorted(wgid, np.arange(NWIN))
        pos_end = np.searchsorted(wgid, np.arange(NWIN), side="right")
        for w in range(NWIN):
            n = pos_end[w] - pos[w]
            idx_all[c, offs[w]:offs[w] + n] = ridx[pos[w]:pos_end[w]]
            dst_all[c, offs[w]:offs[w] + n] = dloc[pos[w]:pos_end[w]]

    chunks = []
    w0 = 0
    while w0 < NWIN:
        w1 = min(w0 + CHUNK_W, NWIN)
        chunks.append((w0, w1, int(offs[w0]), int(offs[w1])))
        w0 = w1
    # compact idx layout: per chunk, [16, (b-a)/16]; concatenated along cols
    idx_sb = []
    for c in range(8):
        cols = []
        for (_, _, a, b) in chunks:
            cols.append(idx_all[c, a:b].reshape(-1, 16).T)
        idx_sb.append(np.concatenate(cols, axis=1))
    idx_sb = np.stack(idx_sb)          # [8, 16, Etot//16]
    dst_sb = np.ascontiguousarray(
        dst_all.reshape(8, -1, P).transpose(0, 2, 1))  # [8, P, Etot//P] int8

    deg = np.zeros(8 * SHARD, np.int64)
    np.add.at(deg, dst, 1)
    xpad = np.zeros((8 * SHARD, F), np.float32)
    xpad[:N_NODES] = x
    bpad = np.full(8 * SHARD, 127, np.float32)
    bpad[:N_NODES] = batch

    in_maps = []
    for c in range(8):
        os_ = slice(c * SHARD, (c + 1) * SHARD)
        in_maps.append({
            "xo_T": np.ascontiguousarray(xpad[os_].T.astype(bf16)),
            "dego": np.ascontiguousarray(
                deg[os_].astype(np.float32).reshape(NW, P).T),
            "batl": np.ascontiguousarray(
                bpad[os_].reshape(NW, P).T.astype(bf16)),
            "idxc": np.ascontiguousarray(idx_sb[c]),
            "dstb": dst_sb[c],
            "W1": np.ascontiguousarray(W1.astype(bf16)),
            "W2": np.ascontiguousarray(W2.astype(bf16)),
            "b1r": np.broadcast_to(b1, (P, F)).astype(bf16).copy(),
            "b2r": np.broadcast_to(b2, (P, F)).astype(bf16).copy(),
            "Wla": _wl_aug(Wl, bl),
        })
    return in_maps, T_w, chunks


def _wl_aug(Wl, bl):
    Wl_aug = np.zeros((F + 1, 4), np.float32)
    Wl_aug[:F, :3] = Wl
    Wl_aug[F, :3] = bl
    Wl_aug[F, 3] = 1.0
    return Wl_aug


def _build(T_w, chunks):
    import concourse.bass as bass
    import concourse.bacc as bacc
    import concourse.tile as tile
    import concourse.mybir as mybir
    from concourse.library_config import mlp
    from concourse.masks import make_identity

    Etot = int(T_w.sum()) * P
    nc = bacc.Bacc("TRN2", target_bir_lowering=False, debug=False,
                   num_devices=8)
    F32, BF, I16, I8 = (mybir.dt.float32, mybir.dt.bfloat16,
                        mybir.dt.int16, mybir.dt.int8)
    AF = mybir.ActivationFunctionType
    OP = mybir.AluOpType

    def ein(name, shape, dt):
        return nc.dram_tensor(name, shape, dt, kind="ExternalInput")

    xo_T = ein("xo_T", [F, SHARD], BF)
    dego = ein("dego", [P, NW], F32)
    batl = ein("batl", [P, NW], BF)
    idxc = ein("idxc", [16, Etot // 16], I16)
    dstb = ein("dstb", [P, Etot // P], I8)
    W1h = ein("W1", [F, F], BF)
    W2h = ein("W2", [F, F], BF)
    b1h = ein("b1r", [P, F], BF)
    b2h = ein("b2r", [P, F], BF)
    Wlh = ein("Wla", [F + 1, 4], F32)
    out_h = nc.dram_tensor("out", [N_GRAPHS, N_ACT], F32,
                           kind="ExternalOutput")

    subt = [nc.dram_tensor(f"sub{i}", [P * (QT + 1), F], F32, kind="Internal")
            for i in range(2)]
    rs_in = [nc.dram_tensor(f"rs_in{i}", [4 * SHARD, F], BF, kind="Internal")
             for i in range(2)]
    rs_out = [nc.dram_tensor(f"rs_out{i}", [SHARD, F], BF, kind="Internal")
              for i in range(2)]
    ag_in = [nc.dram_tensor(f"ag_in{i}", [SHARD, F], BF, kind="Internal")
             for i in range(2)]
    ag_out = [nc.dram_tensor(f"ag_out{i}", [QUART, F], BF, kind="Internal")
              for i in range(2)]
    pool_in = nc.dram_tensor("pool_in", [F + 1, N_GRAPHS], F32,
                             kind="Internal")
    pool_out = nc.dram_tensor("pool_out", [F + 1, N_GRAPHS], F32,
                              kind="Internal", addr_space="Shared")

    RG2 = [[0, 1], [2, 3], [4, 5], [6, 7]]
    RGH = [[0, 2, 4, 6], [1, 3, 5, 7]]
    RG8 = [[0, 1, 2, 3, 4, 5, 6, 7]]

    nc.gpsimd.load_library(mlp)
    with tile.TileContext(nc) as tc:
        with tc.tile_pool(name="cst", bufs=1) as cst, \
             tc.tile_pool(name="big", bufs=1) as big, \
             tc.tile_pool(name="mv", bufs=2) as mv, \
             tc.tile_pool(name="oh", bufs=4) as ohp, \
             tc.tile_pool(name="ps", bufs=2, space="PSUM") as ps, \
             tc.tile_pool(name="pw", bufs=2, space="PSUM") as pw, \
             tc.tile_pool(name="pc", bufs=1, space="PSUM") as pc:

            ident = cst.tile([P, P], BF)
            make_identity(nc, ident[:])
            iota_i = cst.tile([P, P], mybir.dt.int32)
            nc.gpsimd.iota(iota_i[:], pattern=[[1, P]], base=0,
                           channel_multiplier=0)
            iota = cst.tile([P, P], BF)
            nc.vector.tensor_copy(out=iota[:], in_=iota_i[:])

            W1t = cst.tile([F, F], BF)
            nc.sync.dma_start(out=W1t[:], in_=W1h.ap())
            W2t = cst.tile([F, F], BF)
            nc.sync.dma_start(out=W2t[:], in_=W2h.ap())
            b1t = cst.tile([P, F], BF)
            nc.sync.dma_start(out=b1t[:], in_=b1h.ap())
            b2t = cst.tile([P, F], BF)
            nc.sync.dma_start(out=b2t[:], in_=b2h.ap())
            batt = cst.tile([P, NW], BF)
            nc.sync.dma_start(out=batt[:], in_=batl.ap())
            # replicate compact idx across the 8 gpsimd channel groups
            idxt = cst.tile([P, Etot // 16], I16)
            for k in range(8):
                nc.sync.dma_start(out=idxt[16 * k:16 * (k + 1), :],
                                  in_=idxc.ap())
            dst8 = cst.tile([P, Etot // P], I8)
            nc.sync.dma_start(out=dst8[:], in_=dstb.ap())
            dstt = cst.tile([P, Etot // P], BF)
            nc.vector.tensor_copy(out=dstt[:], in_=dst8[:])

            def make_dinv(src_h, n):
                t = cst.tile([P, n], F32, tag=f"dinv{n}")
                nc.sync.dma_start(out=t[:], in_=src_h.ap())
                nc.vector.tensor_scalar(out=t[:], in0=t[:], scalar1=1.0,
                                        scalar2=None, op0=OP.add)
                nc.vector.reciprocal(out=t[:], in_=t[:])
                nc.scalar.activation(t[:], t[:], AF.Sqrt)
                return t

            dinvo = make_dinv(dego, NW)

            stag = big.tile([P, (QT + 1) * F], BF)
            nc.vector.memset(stag[:, QT * F:], 0.0)
            tso = big.tile([P, NW * F], BF)      # tscaled1 own
            h1own = big.tile([P, NW * F], BF)
            self2 = big.tile([P, NW * F], BF)
            h2aug = big.tile([P, NW * (F + 1)], BF)

            s3q = stag[:].rearrange("p (t f) -> p t f", f=F)
            tso3 = tso[:].rearrange("p (t f) -> p t f", f=F)

            # ---- layer 1 transform (own shard), streamed ----
            XC = 28
            for t0 in range(0, NW, XC):
                t1 = min(t0 + XC, NW)
                xc = mv.tile([F, XC * P], BF, tag="xc")
                nc.sync.dma_start(out=xc[:, :(t1 - t0) * P],
                                  in_=xo_T.ap()[:, t0 * P:t1 * P])
                for t in range(t0, t1):
                    pt = pw.tile([P, F], F32, space="PSUM", tag="tr")
                    nc.tensor.matmul(
                        out=pt[:], lhsT=xc[:, (t - t0) * P:(t - t0 + 1) * P],
                        rhs=W1t[:], start=True, stop=True)
                    nc.vector.tensor_tensor(
                        out=tso3[:, t, :], in0=pt[:],
                        in1=dinvo[:, t:t + 1].to_broadcast([P, F]),
                        op=OP.mult)
            nc.sync.dma_start(
                out=ag_in[0].ap().rearrange("(w p) f -> p w f", p=P),
                in_=tso3)
            nc.gpsimd.collective_compute(
                "AllGather", OP.bypass, replica_groups=RG2,
                ins=[ag_in[0].ap()], outs=[ag_out[0].ap()])
            nc.sync.dma_start(
                out=stag[:, :QT * F].rearrange("p (t f) -> p t f", f=F),
                in_=ag_out[0].ap().rearrange("(t p) f -> p t f", p=P))
            nc.gpsimd.dma_start(
                out=subt[0].ap().rearrange("(p t) f -> p t f", p=P),
                in_=stag[:].rearrange("p (t f) -> p t f", f=F))

            MSZ = max((b - a) // P for (_, _, a, b) in chunks)

            def edge_phase(li):
                for (w0, w1, a, b) in chunks:
                    nt = (b - a) // P
                    cpart = mv.tile([P, CHUNK_W * F], BF, tag="cpart")
                    nc.vector.memset(cpart[:], 0.0)
                    cp3 = cpart[:].rearrange("p (w f) -> p w f", f=F)
                    msg = mv.tile([P, MSZ * F], F32, tag="msg")
                    nc.gpsimd.dma_gather(
                        out_ap=msg[:, :nt * F].rearrange(
                            "p (t f) -> p t f", f=F),
                        in_ap=subt[li].ap(),
                        idxs_ap=idxt[:, a // 16:b // 16],
                        num_idxs=b - a,
                        num_idxs_reg=b - a,
                        elem_size=F,
                        single_packet=False,
                    )
                    ti = 0
                    for w in range(w0, w1):
                        tw = int(T_w[w])
                        if tw == 0:
                            continue
                        oht = ohp.tile([P, 8 * P], F32, tag="oh")
                        nc.vector.tensor_tensor(
                            out=oht[:, :tw * P].rearrange(
                                "p (t j) -> p t j", j=P),
                            in0=dstt[:, (a // P) + ti:(a // P) + ti + tw]
                                .unsqueeze(2).to_broadcast([P, tw, P]),
                            in1=iota[:].unsqueeze(1).to_broadcast([P, tw, P]),
                            op=OP.is_equal)
                        acc = ps.tile([P, F], F32, space="PSUM", tag="acc")
                        for k in range(tw):
                            nc.tensor.matmul(
                                out=acc[:],
                                lhsT=oht[:, k * P:(k + 1) * P],
                                rhs=msg[:, (ti + k) * F:(ti + k + 1) * F],
                                start=(k == 0), stop=(k == tw - 1))
                        nc.vector.tensor_copy(out=cp3[:, w - w0, :],
                                              in_=acc[:])
                        ti += tw
                    nc.sync.dma_start(
                        out=rs_in[li].ap()[w0 * P:w1 * P, :].rearrange(
                            "(w p) f -> p w f", p=P),
                        in_=cpart[:, :(w1 - w0) * F].rearrange(
                            "p (w f) -> p w f", f=F))
                nc.gpsimd.collective_compute(
                    "ReduceScatter", OP.add, replica_groups=RGH,
                    ins=[rs_in[li].ap()], outs=[rs_out[li].ap()])

            # ---- layer 1 ----
            edge_phase(0)
            agg1 = big.tile([P, NW * F], BF, tag="agg")
            nc.sync.dma_start(
                out=agg1[:].rearrange("p (w f) -> p w f", f=F),
                in_=rs_out[0].ap().rearrange("(w p) f -> p w f", p=P))
            a3 = agg1[:].rearrange("p (w f) -> p w f", f=F)
            h3 = h1own[:].rearrange("p (w f) -> p w f", f=F)
            # h1 = relu((agg + tscaled1_own) * dinv + b1)
            for w in range(NW):
                dv = dinvo[:, w:w + 1].to_broadcast([P, F])
                nc.vector.tensor_tensor(out=h3[:, w, :], in0=a3[:, w, :],
                                        in1=tso3[:, w, :], op=OP.add)
                nc.vector.tensor_tensor(out=h3[:, w, :], in0=h3[:, w, :],
                                        in1=dv, op=OP.mult)
                nc.vector.tensor_tensor(out=h3[:, w, :], in0=h3[:, w, :],
                                        in1=b1t[:], op=OP.add)
                nc.vector.tensor_scalar(out=h3[:, w, :], in0=h3[:, w, :],
                                        scalar1=0.0, scalar2=None,
                                        op0=OP.max)

            # ---- layer 2 transform (own shard) + self2 ----
            s23 = self2[:].rearrange("p (w f) -> p w f", f=F)
            for w in range(NW):
                trp = pc.tile([P, P], BF, space="PSUM", tag="trp")
                nc.tensor.transpose(out=trp[:F, :], in_=h3[:, w, :],
                                    identity=ident[:])
                h1T = mv.tile([F, P], BF, tag="h1T")
                nc.vector.tensor_copy(out=h1T[:], in_=trp[:F, :])
                pt = pw.tile([P, F], F32, space="PSUM", tag="tr")
                nc.tensor.matmul(out=pt[:], lhsT=h1T[:], rhs=W2t[:],
                                 start=True, stop=True)
                dv = dinvo[:, w:w + 1].to_broadcast([P, F])
                ts2 = mv.tile([P, F], BF, tag="ts2")
                nc.vector.tensor_tensor(out=ts2[:], in0=pt[:], in1=dv,
                                        op=OP.mult)
                nc.vector.tensor_tensor(out=s23[:, w, :], in0=ts2[:], in1=dv,
                                        op=OP.mult)
                nc.sync.dma_start(
                    out=ag_in[1].ap()[w * P:(w + 1) * P, :], in_=ts2[:])
            nc.gpsimd.collective_compute(
                "AllGather", OP.bypass, replica_groups=RG2,
                ins=[ag_in[1].ap()], outs=[ag_out[1].ap()])
            # rebuild staging (bf16) from ag_out, then cast-DMA to subtable2
            nc.sync.dma_start(
                out=stag[:, :QT * F].rearrange("p (t f) -> p t f", f=F),
                in_=ag_out[1].ap().rearrange("(t p) f -> p t f", p=P))
            nc.gpsimd.dma_start(
                out=subt[1].ap().rearrange("(p t) f -> p t f", p=P),
                in_=stag[:].rearrange("p (t f) -> p t f", f=F))

            # ---- layer 2 ----
            edge_phase(1)
            agg2 = big.tile([P, NW * F], BF, tag="agg")
            nc.sync.dma_start(
                out=agg2[:].rearrange("p (w f) -> p w f", f=F),
                in_=rs_out[1].ap().rearrange("(w p) f -> p w f", p=P))
            a23 = agg2[:].rearrange("p (w f) -> p w f", f=F)
            h2a3 = h2aug[:].rearrange("p (w g) -> p w g", g=F + 1)
            nc.vector.memset(h2aug[:], 1.0)
            for w in range(NW):
                dv = dinvo[:, w:w + 1].to_broadcast([P, F])
                nc.vector.tensor_tensor(out=h2a3[:, w, :F], in0=a23[:, w, :],
                                        in1=dv, op=OP.mult)
                nc.vector.tensor_tensor(out=h2a3[:, w, :F],
                                        in0=h2a3[:, w, :F],
                                        in1=s23[:, w, :], op=OP.add)
                nc.vector.tensor_tensor(out=h2a3[:, w, :F],
                                        in0=h2a3[:, w, :F],
                                        in1=b2t[:], op=OP.add)

            # ---- pooling ----
            poolp = pc.tile([F + 1, N_GRAPHS], F32, space="PSUM", tag="pool")
            for w in range(NW):
                ohg = ohp.tile([P, N_GRAPHS], BF, tag="ohg")
                nc.vector.tensor_tensor(
                    out=ohg[:],
                    in0=batt[:, w:w + 1].to_broadcast([P, N_GRAPHS]),
                    in1=iota[:, :N_GRAPHS], op=OP.is_equal)
                nc.tensor.matmul(out=poolp[:], lhsT=h2a3[:, w, :],
                                 rhs=ohg[:], start=(w == 0),
                                 stop=(w == NW - 1))
            pools = cst.tile([F + 1, N_GRAPHS], F32)
            nc.vector.tensor_copy(out=pools[:], in_=poolp[:])
            nc.sync.dma_start(out=pool_in.ap(), in_=pools[:])
            nc.gpsimd.collective_compute(
                "AllReduce", OP.add, replica_groups=RG8,
                ins=[pool_in.ap()], outs=[pool_out.ap()])

            # ---- head ----
            pooled = cst.tile([F + 1, N_GRAPHS], F32)
            nc.sync.dma_start(out=pooled[:], in_=pool_out.ap())
            Wlt = cst.tile([F + 1, 4], F32)
            nc.sync.dma_start(out=Wlt[:], in_=Wlh.ap())
            zp = pc.tile([4, N_GRAPHS], F32, space="PSUM", tag="z")
            nc.tensor.matmul(out=zp[:], lhsT=Wlt[:], rhs=pooled[:],
                             start=True, stop=True)
            zs = cst.tile([4, N_GRAPHS], F32)
            nc.vector.tensor_copy(out=zs[:], in_=zp[:])
            identf = cst.tile([P, P], F32)
            make_identity(nc, identf[:])
            ztp = pc.tile([N_GRAPHS, 4], F32, space="PSUM", tag="zt")
            nc.tensor.transpose(out=ztp[:], in_=zs[:], identity=identf[:4, :4])
            zt = cst.tile([N_GRAPHS, 4], F32)
            nc.vector.tensor_copy(out=zt[:], in_=ztp[:])
            rc = cst.tile([N_GRAPHS, 1], F32)
            nc.vector.reciprocal(out=rc[:], in_=zt[:, 3:4])
            lg = cst.tile([N_GRAPHS, N_ACT], F32)
            nc.vector.tensor_tensor(out=lg[:], in0=zt[:, :N_ACT],
                                    in1=rc[:].to_broadcast([N_GRAPHS, N_ACT]),
                                    op=OP.mult)
            mx = cst.tile([N_GRAPHS, 1], F32)
            nc.vector.tensor_reduce(out=mx[:], in_=lg[:], op=OP.max, axis=mybir.AxisListType.X)
            nc.vector.tensor_tensor(
                out=lg[:], in0=lg[:],
                in1=mx[:].to_broadcast([N_GRAPHS, N_ACT]), op=OP.subtract)
            nc.scalar.activation(lg[:], lg[:], AF.Exp)
            sm = cst.tile([N_GRAPHS, 1], F32)
            nc.vector.tensor_reduce(out=sm[:], in_=lg[:], op=OP.add, axis=mybir.AxisListType.X)
            nc.vector.reciprocal(out=sm[:], in_=sm[:])
            nc.vector.tensor_tensor(
                out=lg[:], in0=lg[:],
                in1=sm[:].to_broadcast([N_GRAPHS, N_ACT]), op=OP.mult)
            nc.sync.dma_start(out=out_h.ap(), in_=lg[:])

    nc.compile()
    return nc


def kernel(x, edge_index, batch, W1, b1, W2, b2, Wl, bl):
    from concourse.bass_utils import run_bass_kernel_spmd
    in_maps, T_w, chunks = _prep(np.asarray(x), np.asarray(edge_index),
                                 np.asarray(batch), np.asarray(W1),
                                 np.asarray(b1), np.asarray(W2),
                                 np.asarray(b2), np.asarray(Wl),
                                 np.asarray(bl))
    nc = _build(T_w, chunks)
    res = run_bass_kernel_spmd(nc, in_maps, core_ids=list(range(8)))
    return np.asarray(res.results[0]["out"], dtype=np.float32)
